# revision 62
# baseline (speedup 1.0000x reference)
"""Trainium2 Bass kernel for nn_MultiHeadAttention_48395691492101.

Head-parallel sharding across 8 NeuronCores (2 heads/core).  Because the
reference reshapes ctx [B,H,T,DV] -> [B,T,H*DV] without transposing, core c
(heads 2c,2c+1) owns output rows [c*256,(c+1)*256) of every batch and needs
no cross-core reduction.

v2 layout (vs the v1 baseline at ~373us TimelineSim):
  - PV stationary is [V | ones] per head, so PSUM rows 64..127 of the
    ctx accumulator hold the softmax row sums replicated across 64
    partitions.  Per-(qg) eviction = Act-engine copy of ctx rows + DVE
    reciprocal of sum rows; normalization is folded into the phase-5
    gather as a bf16 multiply.  The old DRAM-bounce + partition_broadcast
    phase 4 is gone.
  - Software pipelining: during attention(b) the emission stream
    interleaves "filler" PE groups -- output projection of batch b-1 and
    QKV projection / V-transpose of batch b+1 -- so the PE never sits
    behind the exp chain on the Act engine.
  - Output bias via a host-replicated [128,D] tile added on DVE during
    PSUM eviction (dropout multiply on the gpsimd/Pool engine); the K=1
    bias matmuls are gone.
  - PV matmuls on causal-diagonal chunks write only the live column
    subrange (PSUM zero-region accumulation), so the at-tile zero-fills
    are gone.
  - V path, probabilities, Wout, and the phase-5 gather are bf16
    (scores/QK stay f32r); V transposes run at bf16 rate.
  - Host-packed DMA layouts: one descriptor per partition for x, weights,
    Wout; x loads ordered ahead of Wout.
"""

import sys

if "/opt/trn_rl_repo" not in sys.path:
    sys.path.insert(0, "/opt/trn_rl_repo")

import numpy as np

B, T, D = 4, 2048, 1024
H, DK, DV = 16, 64, 64
SCALE = np.float32(1.0 / 8.0)
NCORES = 8
HP = H // NCORES           # heads per core = 2
ROWS = HP * (T * DV) // D  # output rows per head pair per batch = 256
NDC = D // 128             # 8 d-chunks
NTG = 4                    # t-groups of 512 for QKV
NQG = 4                    # q-groups of 512
NKC = T // 128             # 16 k-chunks
MASK_NEG = np.float32(-8.0e9)   # becomes -1e9 after *SCALE inside exp

_cache = {}


def _build(causal: bool = True, debug: bool = False):
    import concourse.tile as tile
    import concourse.mybir as mybir
    from concourse import bacc

    F32 = mybir.dt.float32
    F32R = mybir.dt.float32r
    BF16 = mybir.dt.bfloat16
    F16 = mybir.dt.float16
    Exp = mybir.ActivationFunctionType.Exp

    nc = bacc.Bacc("TRN2", target_bir_lowering=False, debug=False,
                   num_devices=NCORES)

    # host-packed inputs (one contiguous run per partition everywhere);
    # x and the QKV weights are fp16 (scores still accumulate in f32)
    xp_d = nc.dram_tensor("xp", [B * NTG, 128, NDC * 512], F16,
                          kind="ExternalInput").ap()
    wq_d = nc.dram_tensor("wq", [128, D], F16, kind="ExternalInput").ap()
    wkv_d = nc.dram_tensor("wkv", [128, 2 * D], F16,
                           kind="ExternalInput").ap()
    # small f32 pack: bq | bk | bv | dmask(2x128)
    smf_d = nc.dram_tensor("smf", [128, 3 + 256], F32,
                           kind="ExternalInput").ap()
    bvr_d = nc.dram_tensor("bvr", [128, 128], BF16,
                           kind="ExternalInput").ap()
    wout_d = nc.dram_tensor("wout", [128, NDC * D], BF16,
                            kind="ExternalInput").ap()
    bout_d = nc.dram_tensor("boutr", [128, D], F32, kind="ExternalInput").ap()
    drop_d = nc.dram_tensor("drop", [B, ROWS, D], BF16,
                            kind="ExternalInput").ap()
    out_d = nc.dram_tensor("out", [B, ROWS, D], F32, kind="ExternalOutput").ap()

    with tile.TileContext(nc) as tc:
        with tc.tile_pool(name="const", bufs=1) as cpool, \
             tc.tile_pool(name="stream", bufs=2) as stream, \
             tc.tile_pool(name="psum", bufs=1, space="PSUM") as pp:

            # ---- constant loads, packed to minimize HWDGE issue overhead;
            # wq ahead of the first x tiles, everything else behind them ----
            wq_sb = cpool.tile([128, D], F16)
            wkv_sb = cpool.tile([128, 2 * D], F16)
            wk_sb = wkv_sb[:, 0:D]
            wv_sb = wkv_sb[:, D:2 * D]
            smf_sb = cpool.tile([128, 3 + 256], F32)
            bq_sb = smf_sb[:, 0:1]
            bk_sb = smf_sb[:, 1:2]
            bv_sb = smf_sb[:, 2:3]
            dmask_sb = smf_sb[:, 3:259]
            bvrep_sb = cpool.tile([128, 128], BF16)
            nc.sync.dma_start(wq_sb[:], wq_d[:])

            # double-buffered per-batch tensors (explicit pairs)
            wout_sb = cpool.tile([128, NDC * D], BF16)
            bout_sb = cpool.tile([128, D], F32)
            vnb0_sb = cpool.tile([128, NKC * 256], BF16)
            vnb1_sb = cpool.tile([128, NKC * 256], BF16)
            vnb_sb = [vnb0_sb[:], vnb1_sb[:]]
            qt_sb = [cpool.tile([128, T], F32R, name=f"qt{i}")
                     for i in range(2)]
            kt_sb = [cpool.tile([128, T], F32R, name=f"kt{i}")
                     for i in range(2)]
            ex_sb = [cpool.tile([64, 2 * T], BF16, name=f"ex{i}")
                     for i in range(B)]
            rec_sb = [cpool.tile([64, 2 * T], BF16, name=f"rec{i}")
                      for i in range(B)]

            def ones_cols(t):
                return t.rearrange("p (kc h c) -> p kc h c", h=2, c=128)[
                    :, :, :, 64:128]

            def emit_early_consts():
                # all on the in-order sync queue so the HWDGE issue order is
                # exactly this order; vnb ones columns are memset on the idle
                # Pool engine (disjoint from the V evictions, nothing waits)
                nc.sync.dma_start(smf_sb[:], smf_d[:])
                nc.sync.dma_start(bvrep_sb[:], bvr_d[:])
                nc.gpsimd._memset_packed(ones_cols(vnb0_sb), 0x3F80)
                nc.gpsimd._memset_packed(ones_cols(vnb1_sb), 0x3F80)

            def emit_deferred_consts():
                # wout/bout are first needed by the p5(0) fillers in attn(3);
                # issue them on the in-order sync queue AFTER the b1 x tiles
                # so they cannot hog the DMA pipe during phase 1
                nc.sync.dma_start(wout_sb[:], wout_d[:])
                nc.sync.dma_start(bout_sb[:], bout_d[:])

            # ---------------- emission helpers ----------------

            def emit_xt_dmas(b, split_first=False):
                tiles = []
                for tg in range(NTG):
                    xt = stream.tile([128, NDC * 512], F16, tag="xt", bufs=3,
                                     name=f"xt{b}_{tg}")
                    if split_first and tg == 0:
                        hw = NDC * 512 // 2
                        src = xp_d[b * NTG + tg]
                        nc.sync.dma_start(xt[:, 0:hw], src[:, 0:hw])
                        nc.sync.dma_start(xt[:, hw:2 * hw], src[:, hw:2 * hw])
                        emit_early_consts()
                        nc.sync.dma_start(wkv_sb[:], wkv_d[:])
                    else:
                        nc.sync.dma_start(xt[:], xp_d[b * NTG + tg])
                    tiles.append(xt)
                return tiles

            def emit_proj_half(ps, xt, w_sb, lo, bias_sb=None, dst=None,
                               sl=None):
                xv = xt.rearrange("p (dc j) -> p dc j", j=512)
                for dc in range(lo, lo + NDC // 2):
                    nc.tensor.matmul(ps[:], w_sb[:, dc * 128:(dc + 1) * 128],
                                     xv[:, dc, :],
                                     start=(dc == 0), stop=(dc == NDC - 1))
                if dst is not None:
                    nc.vector.tensor_scalar_add(dst[sl], ps[:], bias_sb[:])

            def emit_v_direct(b, tg, tc, xt):
                # V in natural [t, v] orientation: stationary = x chunk,
                # moving = Wv (fp16, ap=128 at full rate) -- no transposes
                kc = 4 * tg + tc
                vnb = vnb_sb[b % 2]
                vnbv = vnb.rearrange("p (kc h c) -> p kc h c", h=2, c=128)
                xv = xt.rearrange("p (dc j) -> p dc j", j=512)
                ps = pp.tile([128, 128], F32, tag="acc", bufs=2,
                             name=f"vd{b}{kc}")
                for dc in range(NDC):
                    nc.tensor.matmul(
                        ps[:], xv[:, dc, tc * 128:(tc + 1) * 128],
                        wv_sb[:, dc * 128:(dc + 1) * 128],
                        start=(dc == 0), stop=(dc == NDC - 1))
                nc.vector.tensor_add(
                    vnbv[:, kc, :, 0:64],
                    ps[:].rearrange("p (h v) -> p h v", h=2),
                    bvrep_sb.rearrange("p (h v) -> p h v", h=2))

            def phase12_jobs(b, split_first=False):
                xts = emit_xt_dmas(b, split_first)
                jobs = []
                state = {}
                tsl = lambda tg: (slice(None), slice(tg * 512, (tg + 1) * 512))
                for tg in range(NTG):
                    specs = [("q", wq_sb, bq_sb, qt_sb[b % 2], tsl(tg)),
                             ("k", wk_sb, bk_sb, kt_sb[b % 2], tsl(tg))]
                    for pn, w_sb, bias_sb, dst, sl in specs:
                        def ja(tg=tg, pn=pn, w_sb=w_sb):
                            ps = pp.tile([128, 512], F32, tag="acc", bufs=2,
                                         name=f"ps{b}{tg}{pn}")
                            state[(tg, pn)] = ps
                            emit_proj_half(ps, xts[tg], w_sb, 0)
                        def jb(tg=tg, pn=pn, w_sb=w_sb, bias_sb=bias_sb,
                               dst=dst, sl=sl):
                            emit_proj_half(state[(tg, pn)], xts[tg], w_sb,
                                           NDC // 2, bias_sb, dst, sl)
                        jobs.append(ja)
                        jobs.append(jb)
                    for tc in range(4):
                        jobs.append(lambda tg=tg, tc=tc: emit_v_direct(
                            b, tg, tc, xts[tg]))
                return jobs

            yts3_sb = [cpool.tile([128, NDC * 128], BF16, name=f"yts3_{i}")
                       for i in range(HP)]

            def p5_gather_job(b, h, pre_yts=None):
                ex = ex_sb[b]
                rec = rec_sb[b]
                if pre_yts is None:
                    yts = stream.tile([128, NDC * 128], BF16, tag="yts",
                                      bufs=2, name=f"yts{b}_{h}")
                    ytv = yts.rearrange("p (c r) -> p c r", r=128)
                    # ex/rec hold the per-head ctx / 1/sum in gather order:
                    # free = two*1024 + s2*128 + r  (t = r*16 + s2*2 + two)
                    ctxv = ex[0:64, h * T:(h + 1) * T]
                    recv = rec[0:64, h * T:(h + 1) * T]
                    for two in range(2):
                        sl = slice(two * 1024, (two + 1) * 1024)
                        nc.vector.tensor_mul(
                            ytv[64 * two:64 * two + 64, :, :],
                            ctxv[:, sl].rearrange("p (s2 r) -> p s2 r", r=128),
                            recv[:, sl].rearrange("p (s2 r) -> p s2 r", r=128))
                else:
                    yts = pre_yts  # partial gathers already emitted per qg
                dt2 = stream.tile([128, D], BF16, tag="dt", bufs=3,
                                  name=f"dt{b}_{h}")
                nc.scalar.dma_start(dt2[:], drop_d[b, h * 128:(h + 1) * 128, :])
                ost = stream.tile([128, D], F32, tag="ost", bufs=3,
                                  name=f"ost{b}_{h}")
                return yts, dt2, ost

            def p5_og_mms(po, yts, og, lo):
                for cc in range(lo, lo + NDC // 2):
                    nc.tensor.matmul(
                        po[:], yts[:, cc * 128:(cc + 1) * 128],
                        wout_sb[:, cc * D + og * 512:cc * D + og * 512 + 512],
                        start=(cc == 0), stop=(cc == NDC - 1))

            def p5_og_evict(b, h, og, dt2, ost, po, last):
                # on the very last group, halve eviction granularity and keep
                # it all on DVE so the end-of-program chain is short;
                # otherwise alternate the dropout multiply between Pool and
                # DVE so neither serializes the ost ring
                for q in ([slice(og * 512, og * 512 + 256),
                           slice(og * 512 + 256, og * 512 + 512)]
                          if last else [slice(og * 512, (og + 1) * 512)]):
                    nc.vector.tensor_add(ost[:, q],
                                         po[:, slice(q.start - og * 512,
                                                     q.stop - og * 512)],
                                         bout_sb[:, q])
                    if last or og == 1:
                        nc.vector.tensor_mul(ost[:, q], ost[:, q], dt2[:, q])
                    else:
                        nc.gpsimd.tensor_mul(ost[:, q], ost[:, q], dt2[:, q])
                    nc.scalar.dma_start(
                        out_d[b, h * 128:(h + 1) * 128, q], ost[:, q])

            def p5_jobs(b, tail=False):
                jobs = []
                state = {}
                for h in range(HP):
                    def gather(b=b, h=h, tail=tail):
                        state[h] = p5_gather_job(
                            b, h, yts3_sb[h] if tail else None)
                    jobs.append(gather)
                    for og in range(2):
                        def oga(b=b, h=h, og=og):
                            po = pp.tile([128, 512], F32, tag="acc", bufs=2,
                                         name=f"po{b}{h}{og}")
                            state[(h, og)] = po
                            p5_og_mms(po, state[h][0], og, 0)
                        def ogb(b=b, h=h, og=og):
                            yts, dt2, ost = state[h]
                            po = state[(h, og)]
                            p5_og_mms(po, yts, og, NDC // 2)
                            p5_og_evict(b, h, og, dt2, ost, po,
                                        tail and h == HP - 1 and og == 1)
                        jobs.append(oga)
                        jobs.append(ogb)
                return jobs

            def emit_attn(b, fillers, eager_gather=False):
                qt, kt = qt_sb[b % 2], kt_sb[b % 2]
                vnb = vnb_sb[b % 2]
                ex = ex_sb[b]
                rec = rec_sb[b]
                dmv = dmask_sb.rearrange("p (h m) -> p h m", h=2)
                niters = sum(4 * qg + 4 for qg in range(NQG))
                njobs = len(fillers)
                popped = 0

                def perm_view(t, h, qg):
                    # [64, two, s2, r'] view of the gather-order layout
                    return t[0:64, h * T:(h + 1) * T].rearrange(
                        "p (two s2 r) -> p two s2 r", two=2, s2=NDC)[
                        :, :, :, qg * 32:(qg + 1) * 32]
                it = 0
                for qg in range(NQG):
                    kcmax = 4 * qg + 4
                    cs2 = pp.tile([128, 1024], F32, tag="cs", bufs=1,
                                  name=f"cs{b}_{qg}")
                    for kc in range(kcmax):
                        o = kc - 4 * qg
                        live = o * 128 if o >= 0 else 0
                        st2 = pp.tile([128, 1024], F32, tag="st", bufs=2,
                                      name=f"st{b}_{qg}_{kc}")
                        stv = st2.rearrange("p (h q) -> p h q", h=2)
                        # o==3 (ap=128) would hit the fp32r short-moving
                        # penalty (4 cyc/row); widening to ap=256 is cheaper
                        mmlive = 256 if live == 384 else live
                        for h in range(2):
                            nc.tensor.matmul(
                                st2[:, h * 512 + mmlive:h * 512 + 512],
                                kt[64 * h:64 * h + 64,
                                   kc * 128:(kc + 1) * 128],
                                qt[64 * h:64 * h + 64,
                                   qg * 512 + mmlive:(qg + 1) * 512],
                                start=True, stop=True)
                        if o >= 0:
                            nc.vector.tensor_add(
                                stv[:, :, live:live + 128],
                                stv[:, :, live:live + 128], dmv[:])
                        at2 = stream.tile([128, 1024], BF16, tag="at", bufs=4,
                                          name=f"at{b}_{qg}_{kc}")
                        atv = at2.rearrange("p (h q) -> p h q", h=2)
                        nc.scalar.activation(atv[:, :, live:512],
                                             stv[:, :, live:512], Exp,
                                             scale=float(SCALE))
                        # filler: keep PE fed while Act runs the exp chain,
                        # paced evenly so supply lasts the whole window
                        while fillers and popped < (it + 1) * njobs // niters:
                            fillers.pop(0)()
                            popped += 1
                        it += 1
                        for h in range(2):
                            nc.tensor.matmul(
                                cs2[:, h * 512 + live:h * 512 + 512],
                                vnb[:, kc * 256 + h * 128:
                                    kc * 256 + h * 128 + 128],
                                at2[:, h * 512 + live:h * 512 + 512],
                                start=(kc == 0), stop=(kc == kcmax - 1))
                    for h in range(2):
                        csl = slice(h * 512, (h + 1) * 512)
                        src = cs2[0:64, csl].rearrange(
                            "p (r s2 two) -> p two s2 r", two=2, s2=NDC)
                        nc.scalar.copy(perm_view(ex, h, qg), src)
                        rsrc = cs2[64:128, csl].rearrange(
                            "p (r s2 two) -> p two s2 r", two=2, s2=NDC)
                        with nc.allow_low_precision("bf16 softmax recip"):
                            nc.vector.reciprocal(perm_view(rec, h, qg), rsrc)
                        if eager_gather:
                            # stream the phase-5 gather per qg so the final
                            # output projection starts almost immediately
                            ytv = yts3_sb[h].rearrange("p (c r) -> p c r",
                                                       r=128)
                            rsl = slice(qg * 32, (qg + 1) * 32)
                            for two in range(2):
                                sl = slice(h * T + two * 1024,
                                           h * T + (two + 1) * 1024)
                                nc.vector.tensor_mul(
                                    ytv[64 * two:64 * two + 64, :, rsl],
                                    ex[0:64, sl].rearrange(
                                        "p (s2 r) -> p s2 r", r=128)
                                    [:, :, rsl],
                                    rec[0:64, sl].rearrange(
                                        "p (s2 r) -> p s2 r", r=128)
                                    [:, :, rsl])
                while fillers:
                    fillers.pop(0)()

            # ---------------- program ----------------
            # b0 phases 1+2 emitted directly; deferred consts after tg0
            b0_jobs = phase12_jobs(0, split_first=True)
            for job in b0_jobs:
                job()

            # p5(b) is deferred so attn(3) -- which has no later QKV work to
            # steal -- gets all three earlier batches' output projections
            f1 = phase12_jobs(1)
            emit_deferred_consts()
            emit_attn(0, f1)
            emit_attn(1, phase12_jobs(2))
            emit_attn(2, phase12_jobs(3))
            emit_attn(3, p5_jobs(0) + p5_jobs(1) + p5_jobs(2),
                      eager_gather=True)

            for job in p5_jobs(B - 1, tail=True):
                job()

    nc.compile()
    return nc


def _get_program():
    if "p" not in _cache:
        _cache["p"] = _build(True)
    return _cache["p"]


def _host_fallback(x, attn_mask, Wq, bq, Wk, bk, Wv, bv, Wout, bout,
                   dropout_mask):
    x64 = x.astype(np.float32)
    Q = np.einsum("btd,hdk->bhtk", x64, Wq) + bq[None, :, None, :]
    K = np.einsum("btd,hdk->bhtk", x64, Wk) + bk[None, :, None, :]
    V = np.einsum("btd,hdv->bhtv", x64, Wv) + bv[None, :, None, :]
    scores = np.einsum("bhqk,bhmk->bhqm", Q, K) * SCALE + attn_mask
    scores = scores - scores.max(-1, keepdims=True)
    e = np.exp(scores)
    attn = e / e.sum(-1, keepdims=True)
    ctx = np.einsum("bhqm,bhmv->bhqv", attn, V).reshape(B, T, H * DV)
    out = ctx @ Wout.T + bout
    return (out * dropout_mask).astype(np.float32)


def kernel(x, attn_mask, Wq, bq, Wk, bk, Wv, bv, Wout, bout, dropout_mask):
    import ml_dtypes
    from concourse.bass_utils import run_bass_kernel_spmd

    BF = ml_dtypes.bfloat16
    x = np.ascontiguousarray(x, np.float32)
    m = np.asarray(attn_mask, np.float32).reshape(T, T)

    # causality check on the actual mask tensor
    causal = bool((np.tril(m) == 0).all() and
                  (m[np.triu_indices(T, 1)] <= -1e8).all())
    if not causal:
        return _host_fallback(x, attn_mask, Wq, bq, Wk, bk, Wv, bv, Wout,
                              bout, dropout_mask)

    # safety: cheap bound on max |scaled score| -> exp overflow guard
    xf = x.reshape(B * T, D)
    Qa = xf @ Wq.transpose(1, 0, 2).reshape(D, H * DK)
    Ka = xf @ Wk.transpose(1, 0, 2).reshape(D, H * DK)
    Qa = Qa.reshape(B * T, H, DK) + bq[None]
    Ka = Ka.reshape(B * T, H, DK) + bk[None]
    qn = np.linalg.norm(Qa, axis=2).max(0)     # per-head max row norm
    kn = np.linalg.norm(Ka, axis=2).max(0)
    bound = float(SCALE) * float((qn * kn).max())
    if bound > 50.0:
        return _host_fallback(x, attn_mask, Wq, bq, Wk, bk, Wv, bv, Wout,
                              bout, dropout_mask)

    nc = _get_program()

    # x packed: [b, tg, p(128), dc(8), j(512)], fp16
    xp = np.ascontiguousarray(
        x.reshape(B, NTG, 512, NDC, 128).transpose(0, 1, 4, 3, 2)
        .reshape(B * NTG, 128, NDC * 512).astype(np.float16))
    woutT = np.asarray(Wout, np.float32).T          # [d', o]
    wout_p = np.ascontiguousarray(
        woutT.reshape(NDC, 128, D).transpose(1, 0, 2)
        .reshape(128, NDC * D).astype(BF))
    bout_rep = np.ascontiguousarray(
        np.broadcast_to(np.asarray(bout, np.float32), (128, D)))
    dmask1 = np.where(np.arange(128)[None, :] < np.arange(128)[:, None],
                      MASK_NEG, np.float32(0.0)).astype(np.float32)
    drop = np.asarray(dropout_mask, np.float32)

    def pack_w(W, h0, h1):
        wc = np.concatenate([W[h0], W[h1]], axis=1)          # [D, 128]
        return (wc.reshape(NDC, 128, 128).transpose(1, 0, 2)
                .reshape(128, D).astype(np.float16))

    in_maps = []
    for c in range(NCORES):
        h0, h1 = HP * c, HP * c + 1
        bqc = np.concatenate([bq[h0], bq[h1]]).reshape(128, 1)
        bkc = np.concatenate([bk[h0], bk[h1]]).reshape(128, 1)
        bvc = np.concatenate([bv[h0], bv[h1]]).reshape(128, 1)
        smf = np.ascontiguousarray(np.concatenate(
            [bqc, bkc, bvc, dmask1, dmask1], axis=1).astype(np.float32))
        bv_rep = np.ascontiguousarray(np.broadcast_to(
            np.concatenate([bv[h0], bv[h1]])[None, :], (128, 128)).astype(BF))
        im = {
            "xp": xp,
            "wq": np.ascontiguousarray(pack_w(Wq, h0, h1)),
            "wkv": np.ascontiguousarray(np.concatenate(
                [pack_w(Wk, h0, h1), pack_w(Wv, h0, h1)], axis=1)),
            "smf": smf,
            "bvr": bv_rep,
            "wout": wout_p,
            "boutr": bout_rep,
            "drop": np.ascontiguousarray(
                drop[:, c * ROWS:(c + 1) * ROWS, :].astype(BF)),
        }
        in_maps.append(im)

    res = run_bass_kernel_spmd(nc, in_maps, list(range(NCORES)))
    out = np.empty((B, T, D), np.float32)
    for c in range(NCORES):
        out[:, c * ROWS:(c + 1) * ROWS, :] = res.results[c]["out"]
    return out
